# revision 2
# baseline (speedup 1.0000x reference)
"""GraphSAGE 2-layer forward on 8 Trainium2 NeuronCores — v3.

Structure (dst-sharded, hint-style halo exchange):
  - 392 dst chunks are re-assigned to (core, slot) by sorted tile count so
    the SPMD slot-max tile profile is tight; the host un-permutes outputs.
  - Layer 2 aggregates h directly (mean-aggregation is linear, so
    mean(h) @ W2_l equals the reference's aggregate-then-project); both
    layers share one edge-slot layout and one drel table.
  - Layer-1 source features are sharded BY EDGE SLOT on the host (the
    per-core input is x already laid out in gather order), so phase A
    streams them with plain linear DMAs.
  - Layer-2 messages are h rows, which only exist on device: h is
    AllGather'd in bf16 (pos-ordered) and gathered per edge with
    dma_gather (int16 indices, lo/hi split on the pos value).
  - One-hot selection matrices are built batched on DVE (one
    tensor_tensor is_equal per chunk, 0-stride broadcast of drel against
    a static iota row-block); mean division is folded in after
    aggregation via a host-precomputed inverse-degree broadcast table.
  - The whole matmul path is bf16 (PE ~107 ns per 128x128x128 tile);
    PSUM accumulation stays fp32.
"""

import sys

sys.path.insert(0, "/opt/trn_rl_repo")

import numpy as np

N = 50000
E = 800000
D_IN, D_HID, D_OUT = 128, 128, 64
N_CORES = 8
CHUNK = 128
C_PER_CORE = 49
NODES_PC = C_PER_CORE * CHUNK  # 6272
NP_ = N_CORES * NODES_PC  # 50176
NCH = N_CORES * C_PER_CORE  # 392
SPLIT = 32768
GROUP_TILES = 56  # tiles per phase-A stream DMA


def _bf16(a):
    import jax.numpy as jnp

    return np.asarray(jnp.asarray(np.asarray(a, np.float32), jnp.bfloat16))


def _preprocess(x, edge_index):
    x = np.asarray(x, dtype=np.float32)
    src = np.asarray(edge_index[0], dtype=np.int64)
    dst = np.asarray(edge_index[1], dtype=np.int64)

    chunk = dst // CHUNK
    n_c = np.bincount(chunk, minlength=NCH)

    # assign chunks to (core, slot): sort by total tile count, deal rows of 8
    t_c = -(-n_c // 128)
    order = np.argsort(-t_c, kind="stable")
    groups = order.reshape(C_PER_CORE, N_CORES)  # groups[j, k] -> chunk id
    core_of = np.empty(NCH, np.int64)
    slot_of = np.empty(NCH, np.int64)
    for j in range(C_PER_CORE):
        for k in range(N_CORES):
            c = groups[j, k]
            core_of[c] = k
            slot_of[c] = j

    # h_full row of node v (h_bounce written in slot order per core)
    v = np.arange(NP_)
    vc = v // CHUNK
    pos = (core_of[vc] * NODES_PC + slot_of[vc] * CHUNK + (v % CHUNK)).astype(
        np.int64
    )

    # edge order: (chunk, pos-hi flag) so each chunk is [lo block | hi block]
    possrc = pos[src]
    hi_flag = (possrc >= SPLIT).astype(np.int64)
    eorder = np.lexsort((hi_flag, chunk))
    ec = chunk[eorder]
    es = src[eorder]
    eps = possrc[eorder]
    ed = dst[eorder]
    ehi = hi_flag[eorder]

    n_lo = np.bincount(ec[ehi == 0], minlength=NCH)
    n_hi = np.bincount(ec[ehi == 1], minlength=NCH)
    tl_c = -(-n_lo // 128)
    th_c = -(-n_hi // 128)
    TL = np.maximum(tl_c[groups].max(axis=1), 1)  # [49]
    TH = th_c[groups].max(axis=1)
    T = TL + TH
    T_total = int(T.sum())
    base = np.zeros(C_PER_CORE, np.int64)
    base[1:] = np.cumsum(T)[:-1]

    # per-edge slot: lo edges fill [0, n_lo) of the chunk's lo region,
    # hi edges fill the hi region starting at TL*128
    starts = np.zeros(NCH + 1, np.int64)
    starts[1:] = np.cumsum(n_c)
    r = np.arange(E) - starts[ec]  # rank within chunk (lo first)
    r_hi = r - n_lo[ec] + TL[slot_of[ec]] * 128  # rank for hi edges
    slot = np.where(ehi == 0, r, r_hi)
    kk = core_of[ec]
    col = base[slot_of[ec]] + slot // 128
    row = slot % 128

    cnt = np.bincount(dst, minlength=NP_).astype(np.float32)
    inv_deg = (1.0 / np.maximum(cnt, 1.0)).astype(np.float32)

    idx_all = np.zeros((N_CORES, 128, T_total), np.int16)
    src_all = np.zeros((N_CORES, 128, T_total), np.int64)
    drel_all = np.full((N_CORES, 128, T_total), 200.0, np.float32)
    idx_all[kk, row, col] = (eps - (ehi * SPLIT)).astype(np.int16)
    src_all[kk, row, col] = es
    drel_all[kk, row, col] = ed % CHUNK

    x_pad = np.zeros((NP_, D_IN), np.float32)
    x_pad[:N] = x

    Tmax = int(T.max())
    iota_big = np.tile(np.arange(128, dtype=np.float32), Tmax)[None, :].repeat(
        128, axis=0
    )
    ident = np.eye(128, dtype=np.float32)

    per_core = []
    for k in range(N_CORES):
        mych = groups[:, k]
        colsel = (mych[:, None] * CHUNK + np.arange(CHUNK)[None, :]).ravel()
        xTb = _bf16(x_pad[colsel].T)  # [128, 6272]
        invb = np.tile(inv_deg[colsel][None, :], (128, 1)).astype(np.float32)
        # layer-1 message stream: x rows in edge-slot order [T_total*128, 128]
        # slot (col t, partition p) -> stream row t*128 + p
        stream = x_pad[src_all[k].T.ravel()]  # [T_total*128, 128]
        # idx table wrapped for dma_gather: [128, T_total*8]
        lin = idx_all[k].T.ravel()  # slot-linear int16 (t*128+p)
        idxw = np.ascontiguousarray(
            np.tile(lin.reshape(T_total * 8, 16).T, (8, 1))
        )
        per_core.append(
            {
                "stream": _bf16(stream),
                "xTb": xTb,
                "idxw": idxw,
                "drel": _bf16(drel_all[k]),
                "invb": np.ascontiguousarray(invb),
                "iotab": _bf16(iota_big),
                "ident": _bf16(ident),
            }
        )

    meta = {"T": [int(t) for t in T], "TL": [int(t) for t in TL],
            "groups": groups}
    return per_core, meta


def _groups_of_slots(T):
    out, cur, tot = [], [], 0
    for j, t in enumerate(T):
        if cur and tot + t > GROUP_TILES:
            out.append(cur)
            cur, tot = [], 0
        cur.append(j)
        tot += t
    if cur:
        out.append(cur)
    return out


def _build(meta):
    import concourse.bacc as bacc
    import concourse.mybir as mybir
    from concourse.tile import TileContext

    T = meta["T"]
    TL = meta["TL"]
    f32 = mybir.dt.float32
    bf16 = mybir.dt.bfloat16
    i16 = mybir.dt.int16
    T_total = sum(T)
    Tmax = max(T)
    base = np.zeros(len(T), np.int64)
    base[1:] = np.cumsum(T)[:-1]
    sgroups = _groups_of_slots(T)
    GTmax = max(sum(T[j] for j in g) for g in sgroups)

    nc = bacc.Bacc(
        "TRN2",
        target_bir_lowering=False,
        debug=False,
        enable_asserts=False,
        num_devices=N_CORES,
    )

    stream_d = nc.dram_tensor(
        "stream", [T_total * 128, D_IN], bf16, kind="ExternalInput"
    ).ap()
    xTb_d = nc.dram_tensor("xTb", [128, NODES_PC], bf16, kind="ExternalInput").ap()
    idxw_d = nc.dram_tensor(
        "idxw", [128, T_total * 8], i16, kind="ExternalInput"
    ).ap()
    drel_d = nc.dram_tensor("drel", [128, T_total], bf16, kind="ExternalInput").ap()
    invb_d = nc.dram_tensor("invb", [128, NODES_PC], f32, kind="ExternalInput").ap()
    iotab_d = nc.dram_tensor(
        "iotab", [128, Tmax * 128], bf16, kind="ExternalInput"
    ).ap()
    ident_d = nc.dram_tensor("ident", [128, 128], bf16, kind="ExternalInput").ap()
    w1l_d = nc.dram_tensor("W1_l", [D_IN, D_HID], bf16, kind="ExternalInput").ap()
    w1r_d = nc.dram_tensor("W1_r", [D_IN, D_HID], bf16, kind="ExternalInput").ap()
    w2l_d = nc.dram_tensor("W2_l", [D_HID, D_OUT], bf16, kind="ExternalInput").ap()
    w2r_d = nc.dram_tensor("W2_r", [D_HID, D_OUT], bf16, kind="ExternalInput").ap()
    b1r_d = nc.dram_tensor("b1r", [1, D_HID], bf16, kind="ExternalInput").ap()
    b2_d = nc.dram_tensor("b2", [1, D_OUT], bf16, kind="ExternalInput").ap()
    out_d = nc.dram_tensor("out", [NODES_PC, D_OUT], f32, kind="ExternalOutput").ap()
    h_bounce_d = nc.dram_tensor(
        "h_bounce", [NODES_PC, D_HID], bf16, kind="Internal"
    ).ap()
    h_full = nc.dram_tensor(
        "h_full", [NP_, D_HID], bf16, kind="Internal", addr_space="Shared"
    ).ap()

    relu = mybir.ActivationFunctionType.Relu
    is_eq = mybir.AluOpType.is_equal
    mult = mybir.AluOpType.mult

    with TileContext(nc) as tc:
        with (
            tc.tile_pool(name="persist", bufs=1) as pp,
            tc.tile_pool(name="msg", bufs=2) as mpool,
            tc.tile_pool(name="oh", bufs=3) as ohpool,
            tc.tile_pool(name="stage", bufs=4) as spool,
            tc.tile_pool(name="psA", bufs=2, space="PSUM") as psA,
            tc.tile_pool(name="psH", bufs=2, space="PSUM") as psH,
            tc.tile_pool(name="psT", bufs=2, space="PSUM") as psT,
        ):
            xTb_sb = pp.tile([128, NODES_PC], bf16)
            nc.sync.dma_start(out=xTb_sb[:], in_=xTb_d)
            idxw_sb = pp.tile([128, T_total * 8], i16)
            nc.sync.dma_start(out=idxw_sb[:], in_=idxw_d)
            drel_sb = pp.tile([128, T_total], bf16)
            nc.sync.dma_start(out=drel_sb[:], in_=drel_d)
            invb_sb = pp.tile([128, NODES_PC], f32)
            nc.sync.dma_start(out=invb_sb[:], in_=invb_d)
            iota_sb = pp.tile([128, Tmax * 128], bf16)
            nc.sync.dma_start(out=iota_sb[:], in_=iotab_d)
            ident_sb = pp.tile([128, 128], bf16)
            nc.sync.dma_start(out=ident_sb[:], in_=ident_d)
            w1l_sb = pp.tile([D_IN, D_HID], bf16)
            nc.sync.dma_start(out=w1l_sb[:], in_=w1l_d)
            w1r_sb = pp.tile([D_IN, D_HID], bf16)
            nc.sync.dma_start(out=w1r_sb[:], in_=w1r_d)
            w2l_sb = pp.tile([D_HID, D_OUT], bf16)
            nc.sync.dma_start(out=w2l_sb[:], in_=w2l_d)
            w2r_sb = pp.tile([D_HID, D_OUT], bf16)
            nc.sync.dma_start(out=w2r_sb[:], in_=w2r_d)
            b1r_sb = pp.tile([1, D_HID], bf16)
            nc.sync.dma_start(out=b1r_sb[:], in_=b1r_d)
            b2_sb = pp.tile([1, D_OUT], bf16)
            nc.sync.dma_start(out=b2_sb[:], in_=b2_d)
            ones_sb = pp.tile([1, 128], bf16)
            nc.vector.memset(ones_sb[:], 1.0)
            h_all = pp.tile([128, NODES_PC], bf16)  # h^T (feature-major)

            def build_oh(j, tj):
                oh = ohpool.tile([128, Tmax * 128], bf16, tag="oh")
                nc.vector.tensor_tensor(
                    out=oh[:, : tj * 128].rearrange("p (t d) -> p t d", d=128),
                    in0=drel_sb[:, base[j] : base[j] + tj].to_broadcast(
                        [128, tj, 128]
                    ),
                    in1=iota_sb[:, : tj * 128].rearrange("p (t d) -> p t d", d=128),
                    op=is_eq,
                )
                return oh

            def agg_mean(msg, oh, j, tj, tb):
                pa = psA.tile([128, 128], f32, tag="agg")
                for t in range(tj):
                    nc.tensor.matmul(
                        out=pa[:],
                        lhsT=msg[:, (tb + t) * 128 : (tb + t + 1) * 128],
                        rhs=oh[:, t * 128 : (t + 1) * 128],
                        start=(t == 0),
                        stop=(t == tj - 1),
                    )
                jsl = slice(j * 128, (j + 1) * 128)
                meanT = spool.tile([128, 128], bf16, tag="meanT")
                nc.vector.tensor_tensor(
                    out=meanT[:], in0=pa[:], in1=invb_sb[:, jsl], op=mult
                )
                return meanT

            # ---------------- phase A: layer 1 -> h ----------------
            for g in sgroups:
                gt = int(sum(T[j] for j in g))
                b0 = int(base[g[0]])
                msg = mpool.tile([128, GTmax * 128], bf16, tag="msg")
                nc.sync.dma_start(
                    out=msg[:, : gt * 128].rearrange("p (t d) -> p t d", d=D_IN),
                    in_=stream_d[b0 * 128 : (b0 + gt) * 128, :].rearrange(
                        "(t p) d -> p t d", p=128
                    ),
                )
                for j in g:
                    tj = int(T[j])
                    tb = int(base[j]) - b0
                    oh = build_oh(j, tj)
                    meanT = agg_mean(msg, oh, j, tj, tb)
                    jsl = slice(j * 128, (j + 1) * 128)
                    ph = psH.tile([128, 128], f32, tag="h")
                    nc.tensor.matmul(
                        out=ph[:], lhsT=meanT[:], rhs=w1l_sb[:], start=True, stop=False
                    )
                    nc.tensor.matmul(
                        out=ph[:],
                        lhsT=xTb_sb[:, jsl],
                        rhs=w1r_sb[:],
                        start=False,
                        stop=False,
                    )
                    # h row-major [dst, hid]: b1 varies along the free dim
                    # here, so fold it in with a ones x b1-row matmul.
                    nc.tensor.matmul(
                        out=ph[:],
                        lhsT=ones_sb[:],
                        rhs=b1r_sb[:],
                        start=False,
                        stop=True,
                    )
                    h_sb = spool.tile([128, 128], bf16, tag="h_sb")
                    nc.scalar.activation(
                        out=h_sb[:], in_=ph[:], func=relu, scale=1.0
                    )
                    nc.sync.dma_start(out=h_bounce_d[jsl, :], in_=h_sb[:])
                    # h^T for the layer-2 root term
                    pt = psT.tile([128, 128], f32, tag="hT")
                    nc.tensor.matmul(
                        out=pt[:], lhsT=h_sb[:], rhs=ident_sb[:], start=True, stop=True
                    )
                    nc.scalar.copy(out=h_all[:, jsl], in_=pt[:])

            # ---------------- all-gather h ----------------
            nc.gpsimd.collective_compute(
                "AllGather",
                mybir.AluOpType.bypass,
                replica_groups=[list(range(N_CORES))],
                ins=[h_bounce_d],
                outs=[h_full],
            )

            # ---------------- phase B: layer 2 ----------------
            tbg = 0
            for j in range(C_PER_CORE):
                tj = int(T[j])
                tlj = int(TL[j])
                thj = tj - tlj
                bj = int(base[j])
                msg = mpool.tile([128, Tmax * 128], bf16, tag="msg2")
                if tlj:
                    nc.gpsimd.dma_gather(
                        out_ap=msg[:, : tlj * 128].rearrange(
                            "p (t d) -> p t d", d=D_HID
                        ),
                        in_ap=h_full[0:SPLIT, :],
                        idxs_ap=idxw_sb[:, bj * 8 : (bj + tlj) * 8],
                        num_idxs=tlj * 128,
                        num_idxs_reg=tlj * 128,
                        elem_size=D_HID,
                        single_packet=False,
                    )
                if thj:
                    nc.gpsimd.dma_gather(
                        out_ap=msg[:, tlj * 128 : tj * 128].rearrange(
                            "p (t d) -> p t d", d=D_HID
                        ),
                        in_ap=h_full[SPLIT:NP_, :],
                        idxs_ap=idxw_sb[:, (bj + tlj) * 8 : (bj + tj) * 8],
                        num_idxs=thj * 128,
                        num_idxs_reg=thj * 128,
                        elem_size=D_HID,
                        single_packet=False,
                    )
                oh = build_oh(j, tj)
                meanhT = agg_mean(msg, oh, j, tj, 0)
                jsl = slice(j * 128, (j + 1) * 128)
                pf = psH.tile([128, D_OUT], f32, tag="fin")
                nc.tensor.matmul(
                    out=pf[:], lhsT=meanhT[:], rhs=w2l_sb[:], start=True, stop=False
                )
                nc.tensor.matmul(
                    out=pf[:],
                    lhsT=h_all[:, jsl],
                    rhs=w2r_sb[:],
                    start=False,
                    stop=False,
                )
                nc.tensor.matmul(
                    out=pf[:], lhsT=ones_sb[:], rhs=b2_sb[:], start=False, stop=True
                )
                out_sb = spool.tile([128, D_OUT], f32, tag="out_sb")
                nc.scalar.copy(out=out_sb[:], in_=pf[:])
                nc.sync.dma_start(out=out_d[jsl, :], in_=out_sb[:])

    nc.compile()
    return nc


def _shared_inputs(inp):
    return {
        "W1_l": _bf16(inp["W1_l"]),
        "W1_r": _bf16(inp["W1_r"]),
        "W2_l": _bf16(inp["W2_l"]),
        "W2_r": _bf16(inp["W2_r"]),
        "b1r": _bf16(np.asarray(inp["b1"], np.float32).reshape(1, D_HID)),
        "b2": _bf16(np.asarray(inp["b2"], np.float32).reshape(1, D_OUT)),
    }


def kernel(x, edge_index, W1_l, b1, W1_r, W2_l, b2, W2_r):
    from concourse.bass_utils import run_bass_kernel_spmd

    per_core, meta = _preprocess(x, edge_index)
    nc = _build(meta)

    shared = _shared_inputs(
        {"W1_l": W1_l, "W1_r": W1_r, "W2_l": W2_l, "W2_r": W2_r, "b1": b1, "b2": b2}
    )
    in_maps = [{**pc, **shared} for pc in per_core]

    res = run_bass_kernel_spmd(nc, in_maps, core_ids=list(range(N_CORES)))
    groups = meta["groups"]
    out_full = np.empty((NP_, D_OUT), np.float32)
    for k in range(N_CORES):
        ok = np.asarray(res.results[k]["out"], np.float32)
        for j in range(C_PER_CORE):
            c = int(groups[j, k])
            out_full[c * CHUNK : (c + 1) * CHUNK] = ok[j * CHUNK : (j + 1) * CHUNK]
    return out_full[:N].astype(np.float32)


if __name__ == "__main__":
    rng = np.random.default_rng(0)
    x = rng.standard_normal((N, D_IN), dtype=np.float32)
    ei = rng.integers(0, N, size=(2, E), dtype=np.int64)
    s = 1.0 / np.sqrt(D_IN)
    out = kernel(
        x=x,
        edge_index=ei,
        W1_l=rng.uniform(-s, s, (D_IN, D_HID)).astype(np.float32),
        b1=np.zeros(D_HID, np.float32),
        W1_r=rng.uniform(-s, s, (D_IN, D_HID)).astype(np.float32),
        W2_l=rng.uniform(-s, s, (D_HID, D_OUT)).astype(np.float32),
        b2=np.zeros(D_OUT, np.float32),
        W2_r=rng.uniform(-s, s, (D_HID, D_OUT)).astype(np.float32),
    )
    print(out.shape, out.dtype)
